# revision 5
# baseline (speedup 1.0000x reference)
"""Multi-head attention (B=2, T=2048, H=8, K=128) on 8 TRN2 NeuronCores.

Sharding: tensor-parallel over heads — core c owns head c for both batches.
Each core computes its head's attention output projected through its slice
of Wu (a partial sum over the unified dim); the host sums the 8 partials
and adds the bias.

Per-core dataflow (everything "transposed": features on partitions, tokens
on the moving/free axis):
  X^T  [k=128, t=4096]   via PE transposes of 32 [128,128] tiles of x
  Q^T  = Wq_h^T X^T      [i=128, 4096]  fp32r matmul, evicted as bf16
  K^T  = Wk_h^T X^T      [i=128, 4096]  fp32r matmul, evicted as bf16
  V^T  = Wv_h^T X^T      [j=128, 4096]  fp32r, PE-transposed back to
  V    [s=128-chunks, j=128] bf16       (lhsT for the Y^T matmul)
  per 1024-token block t:
    per 128-key chunk s:
      S^T_s = K_s Q^T            [128, 1024] PSUM   (bf16 matmul, fp32 acc)
      E_s   = exp(S^T_s/sqrt(128))  ACT, PSUM -> SBUF bf16
      sumexp += ones^T E_s       [128, 1024] PSUM   (replicated over parts)
      Y^T   += V_s^T E_s         [128, 1024] PSUM
    Y^T_norm = Y^T * recip(sumexp)   DVE, -> SBUF fp32r
  out^T = Wu_h^T Y^T_norm   [o=128, 4096] fp32r -> DRAM

bf16 is used only for the T×T-scale attention matmuls (rounding error
~1e-3 on softmax weights); projections, softmax statistics, normalization
and the output projection stay fp32(r).

Host: out = sum_c out_c^T.T + bu, reshaped to (2, 2048, 128).
"""

import sys

import numpy as np

if "/opt/trn_rl_repo" not in sys.path:
    sys.path.insert(0, "/opt/trn_rl_repo")

B, T, K, H = 2, 2048, 128, 8
BT = B * T              # 4096 tokens over both batches
NT = BT // 128          # 32 token tiles of 128
NCORES = 8
TB = 1024               # token block (2 psum banks)
NS = T // 128           # 16 key chunks per batch
SCALE = 1.0 / np.sqrt(np.float32(K))

_compiled = None


def _build():
    import concourse.mybir as mybir
    import concourse.tile as tile
    from concourse import bacc
    from concourse.masks import make_identity

    f32 = mybir.dt.float32
    f32r = mybir.dt.float32r
    bf16 = mybir.dt.bfloat16
    Exp = mybir.ActivationFunctionType.Exp

    nc = bacc.Bacc(
        "TRN2",
        target_bir_lowering=False,
        debug=False,
        enable_asserts=False,
        num_devices=NCORES,
    )

    x_d = nc.dram_tensor("x", [BT, K], f32, kind="ExternalInput").ap()
    wq_d = nc.dram_tensor("wq", [K, K], f32, kind="ExternalInput").ap()
    wk_d = nc.dram_tensor("wk", [K, K], f32, kind="ExternalInput").ap()
    wv_d = nc.dram_tensor("wv", [K, K], f32, kind="ExternalInput").ap()
    wu_d = nc.dram_tensor("wu", [K, K], f32, kind="ExternalInput").ap()
    out_d = nc.dram_tensor("out", [K, BT], f32, kind="ExternalOutput").ap()

    with tile.TileContext(nc) as tc:
        from contextlib import ExitStack

        with ExitStack() as ctx:
            const = ctx.enter_context(tc.tile_pool(name="const", bufs=1))
            big = ctx.enter_context(tc.tile_pool(name="big", bufs=1))
            work = ctx.enter_context(tc.tile_pool(name="work", bufs=3))
            # PSUM budget (8 banks): s 2x[128,1024] = 4, y 1x = 2, sum 1x = 2
            ps_s = ctx.enter_context(tc.tile_pool(name="ps_s", bufs=2, space="PSUM"))
            ps_y = ctx.enter_context(tc.tile_pool(name="ps_y", bufs=1, space="PSUM"))
            ps_sum = ctx.enter_context(tc.tile_pool(name="ps_sum", bufs=1, space="PSUM"))

            ident = const.tile([128, 128], f32)
            make_identity(nc, ident[:])
            ones = const.tile([128, 128], bf16)
            nc.gpsimd.memset(ones[:], 1.0)

            # weights: DMA fp32, then DVE cast-copy to fp32r (the verifier
            # requires fp32r matmul operands to be rounded on write)
            wq_st = const.tile([128, 128], f32, tag="wq_st")
            wk_st = const.tile([128, 128], f32, tag="wk_st")
            wv_st = const.tile([128, 128], f32, tag="wv_st")
            wu_st = const.tile([128, 128], f32, tag="wu_st")
            nc.sync.dma_start(wq_st[:], wq_d[:])
            nc.sync.dma_start(wk_st[:], wk_d[:])
            nc.sync.dma_start(wv_st[:], wv_d[:])
            nc.sync.dma_start(wu_st[:], wu_d[:])
            wq_sb = const.tile([128, 128], f32r, tag="wq")
            wk_sb = const.tile([128, 128], f32r, tag="wk")
            wv_sb = const.tile([128, 128], f32r, tag="wv")
            wu_sb = const.tile([128, 128], f32r, tag="wu")
            nc.vector.tensor_copy(wq_sb[:], wq_st[:])
            nc.vector.tensor_copy(wk_sb[:], wk_st[:])
            nc.vector.tensor_copy(wv_sb[:], wv_st[:])
            nc.vector.tensor_copy(wu_sb[:], wu_st[:])

            # x, tiled [p=128, n=32, k=128]; x_sb[p, n, k] = x[n*128+p, k]
            x_sb = big.tile([128, NT, 128], f32, tag="x")
            x_re = x_d.rearrange("(n p) k -> p n k", p=128)
            for h in range(4):
                nc.sync.dma_start(x_sb[:, 8 * h : 8 * (h + 1), :],
                                  x_re[:, 8 * h : 8 * (h + 1), :])

            # X^T [k, t] (fp32r: feeds the fp32r projections)
            xt = big.tile([128, BT], f32r, tag="xt")
            for n in range(NT):
                pt = ps_s.tile([128, 128], f32, tag="s")
                nc.tensor.transpose(pt[:], x_sb[:, n, :], ident[:])
                nc.vector.tensor_copy(xt[:, 128 * n : 128 * (n + 1)], pt[:])

            # projections (fp32r), evicted to bf16 for the attention matmuls
            qt = big.tile([128, BT], bf16, tag="qt")
            kt = big.tile([128, BT], bf16, tag="kt")
            vt = big.tile([128, BT], f32, tag="vt")
            for w_sb, dst in ((wq_sb, qt), (wk_sb, kt), (wv_sb, vt)):
                for blk in range(BT // 512):
                    pp = ps_s.tile([128, 512], f32, tag="s")
                    nc.tensor.matmul(
                        pp[:],
                        w_sb[:],
                        xt[:, 512 * blk : 512 * (blk + 1)],
                        start=True,
                        stop=True,
                    )
                    nc.vector.tensor_copy(dst[:, 512 * blk : 512 * (blk + 1)], pp[:])

            # V back to [s, j] layout (bf16), chunk c at cols c*128
            v_sb = big.tile([128, BT], bf16, tag="v")
            for c in range(NT):
                pt = ps_s.tile([128, 128], f32, tag="s")
                nc.tensor.transpose(pt[:], vt[:, 128 * c : 128 * (c + 1)], ident[:])
                nc.vector.tensor_copy(v_sb[:, 128 * c : 128 * (c + 1)], pt[:])

            # attention
            y_sb = big.tile([128, BT], f32r, tag="y")
            for b in range(B):
                for tb in range(T // TB):
                    tcol = b * T + tb * TB
                    py = ps_y.tile([128, TB], f32, tag="y")
                    psumt = ps_sum.tile([128, TB], f32, tag="sum")
                    for s in range(NS):
                        scol = b * T + s * 128
                        ps = ps_s.tile([128, TB], f32, tag="s")
                        for g in range(TB // 512):
                            sl = slice(512 * g, 512 * (g + 1))
                            nc.tensor.matmul(
                                ps[:, sl],
                                kt[:, scol : scol + 128],
                                qt[:, tcol + 512 * g : tcol + 512 * (g + 1)],
                                start=True,
                                stop=True,
                            )
                        e_sb = work.tile([128, TB], bf16, tag="e")
                        nc.scalar.activation(e_sb[:], ps[:], Exp, scale=float(SCALE))
                        for g in range(TB // 512):
                            sl = slice(512 * g, 512 * (g + 1))
                            nc.tensor.matmul(
                                psumt[:, sl],
                                ones[:],
                                e_sb[:, sl],
                                start=(s == 0),
                                stop=(s == NS - 1),
                                skip_group_check=True,
                            )
                            nc.tensor.matmul(
                                py[:, sl],
                                v_sb[:, scol : scol + 128],
                                e_sb[:, sl],
                                start=(s == 0),
                                stop=(s == NS - 1),
                                skip_group_check=True,
                            )
                    r_sb = work.tile([128, TB], f32, tag="r")
                    nc.vector.reciprocal(r_sb[:], psumt[:])
                    nc.vector.tensor_mul(y_sb[:, tcol : tcol + TB], py[:], r_sb[:])

            # unify: out^T = Wu_h^T @ Y^T  (fp32r)
            out_sb = big.tile([128, BT], f32, tag="out")
            for blk in range(BT // 512):
                po = ps_s.tile([128, 512], f32, tag="s")
                nc.tensor.matmul(
                    po[:],
                    wu_sb[:],
                    y_sb[:, 512 * blk : 512 * (blk + 1)],
                    start=True,
                    stop=True,
                )
                nc.vector.tensor_copy(out_sb[:, 512 * blk : 512 * (blk + 1)], po[:])
                nc.sync.dma_start(out_d[:, 512 * blk : 512 * (blk + 1)],
                                  out_sb[:, 512 * blk : 512 * (blk + 1)])

    nc.compile()
    return nc


def _get_nc():
    global _compiled
    if _compiled is None:
        _compiled = _build()
    return _compiled


def kernel(x, Wq, Wk, Wv, Wu, bu, **_run_kwargs):
    from concourse.bass_utils import run_bass_kernel_spmd

    nc = _get_nc()

    x = np.ascontiguousarray(np.asarray(x, dtype=np.float32).reshape(BT, K))
    Wq = np.asarray(Wq, dtype=np.float32)
    Wk = np.asarray(Wk, dtype=np.float32)
    Wv = np.asarray(Wv, dtype=np.float32)
    Wu = np.asarray(Wu, dtype=np.float32)
    bu = np.asarray(bu, dtype=np.float32)

    in_maps = []
    for c in range(NCORES):
        sl = slice(c * K, (c + 1) * K)
        in_maps.append(
            {
                "x": x,
                "wq": np.ascontiguousarray(Wq[:, sl]),
                "wk": np.ascontiguousarray(Wk[:, sl]),
                "wv": np.ascontiguousarray(Wv[:, sl]),
                "wu": np.ascontiguousarray(Wu[sl, :]),
            }
        )

    res = run_bass_kernel_spmd(nc, in_maps, list(range(NCORES)), **_run_kwargs)

    out = np.zeros((BT, K), dtype=np.float32)
    for c in range(NCORES):
        out += res.results[c]["out"].T
    out += bu[None, :]
    result = out.reshape(B, T, K)
    if _run_kwargs:
        return result, res
    return result


# revision 9
# speedup vs baseline: 1.1494x; 1.1494x over previous
"""Multi-head attention (B=2, T=2048, H=8, K=128) on 8 TRN2 NeuronCores.

Sharding: tensor-parallel over heads — core c owns head c for both batches.
Each core computes its head's attention output projected through its slice
of Wu (a partial sum over the unified dim); the host sums the 8 partials
and adds the bias.

Per-core dataflow (everything "transposed": features on partitions, tokens
on the moving/free axis):
  X^T  [k=128, t=4096]   via PE transposes of 32 [128,128] tiles of x
  Q^T  = Wq_h^T X^T      [i=128, 4096]  fp32r matmul, evicted as bf16
  K^T  = Wk_h^T X^T      [i=128, 4096]  fp32r matmul, evicted as bf16
  V^T  = Wv_h^T X^T      [j=128, 4096]  fp32r, PE-transposed back to
  V    [s=128-chunks, j=128] bf16       (lhsT for the Y^T matmul)
  per 1024-token block t:
    per 128-key chunk s:
      S^T_s = K_s Q^T            [128, 1024] PSUM   (bf16 matmul, fp32 acc)
      E_s   = exp(S^T_s/sqrt(128))  ACT, PSUM -> SBUF bf16
      sumexp += ones^T E_s       [128, 1024] PSUM   (replicated over parts)
      Y^T   += V_s^T E_s         [128, 1024] PSUM
    Y^T_norm = Y^T * recip(sumexp)   DVE, -> SBUF fp32r
  out^T = Wu_h^T Y^T_norm   [o=128, 4096] fp32r -> DRAM

bf16 is used only for the T×T-scale attention matmuls (rounding error
~1e-3 on softmax weights); projections, softmax statistics, normalization
and the output projection stay fp32(r).

Host: out = sum_c out_c^T.T + bu, reshaped to (2, 2048, 128).
"""

import sys

import numpy as np

if "/opt/trn_rl_repo" not in sys.path:
    sys.path.insert(0, "/opt/trn_rl_repo")

B, T, K, H = 2, 2048, 128, 8
BT = B * T              # 4096 tokens over both batches
NT = BT // 128          # 32 token tiles of 128
NCORES = 8
TB = 1024               # token block (2 psum banks)
NS = T // 128           # 16 key chunks per batch
SCALE = 1.0 / np.sqrt(np.float32(K))

_compiled = None


def _build():
    import concourse.mybir as mybir
    import concourse.tile as tile
    from concourse import bacc
    from concourse.masks import make_identity

    f32 = mybir.dt.float32
    f32r = mybir.dt.float32r
    bf16 = mybir.dt.bfloat16
    Exp = mybir.ActivationFunctionType.Exp

    nc = bacc.Bacc(
        "TRN2",
        target_bir_lowering=False,
        debug=False,
        enable_asserts=False,
        num_devices=NCORES,
    )

    x_d = nc.dram_tensor("x", [BT, K], f32, kind="ExternalInput").ap()
    wq_d = nc.dram_tensor("wq", [K, K], f32, kind="ExternalInput").ap()
    wk_d = nc.dram_tensor("wk", [K, K], f32, kind="ExternalInput").ap()
    wv_d = nc.dram_tensor("wv", [K, K], f32, kind="ExternalInput").ap()
    wu_d = nc.dram_tensor("wu", [K, K], f32, kind="ExternalInput").ap()
    out_d = nc.dram_tensor("out", [K, BT], f32, kind="ExternalOutput").ap()

    with tile.TileContext(nc) as tc:
        from contextlib import ExitStack

        with ExitStack() as ctx:
            const = ctx.enter_context(tc.tile_pool(name="const", bufs=1))
            big = ctx.enter_context(tc.tile_pool(name="big", bufs=1))
            work = ctx.enter_context(tc.tile_pool(name="work", bufs=3))
            # PSUM budget (8 banks): s 2x[128,1024] = 4, y 1x = 2, sum 1x = 2
            ps_s = ctx.enter_context(tc.tile_pool(name="ps_s", bufs=2, space="PSUM"))
            ps_y = ctx.enter_context(tc.tile_pool(name="ps_y", bufs=1, space="PSUM"))
            ps_sum = ctx.enter_context(tc.tile_pool(name="ps_sum", bufs=1, space="PSUM"))

            ident = const.tile([128, 128], f32)
            make_identity(nc, ident[:])
            ones = const.tile([128, 128], bf16)
            nc.gpsimd.memset(ones[:], 1.0)

            # weights: DMA fp32, then DVE cast-copy to fp32r (the verifier
            # requires fp32r matmul operands to be rounded on write)
            wq_st = const.tile([128, 128], f32, tag="wq_st")
            wk_st = const.tile([128, 128], f32, tag="wk_st")
            wv_st = const.tile([128, 128], f32, tag="wv_st")
            wu_st = const.tile([128, 128], f32, tag="wu_st")
            nc.sync.dma_start(wq_st[:], wq_d[:])
            nc.sync.dma_start(wk_st[:], wk_d[:])
            nc.sync.dma_start(wv_st[:], wv_d[:])
            nc.sync.dma_start(wu_st[:], wu_d[:])
            wq_sb = const.tile([128, 128], f32r, tag="wq")
            wk_sb = const.tile([128, 128], f32r, tag="wk")
            wv_sb = const.tile([128, 128], f32r, tag="wv")
            wu_sb = const.tile([128, 128], f32r, tag="wu")
            nc.vector.tensor_copy(wq_sb[:], wq_st[:])
            nc.vector.tensor_copy(wk_sb[:], wk_st[:])
            nc.vector.tensor_copy(wv_sb[:], wv_st[:])
            nc.vector.tensor_copy(wu_sb[:], wu_st[:])

            # x, tiled [p=128, n=32, k=128]; x_sb[p, n, k] = x[n*128+p, k]
            x_sb = big.tile([128, NT, 128], f32, tag="x")
            x_re = x_d.rearrange("(n p) k -> p n k", p=128)
            for h in range(4):
                nc.sync.dma_start(x_sb[:, 8 * h : 8 * (h + 1), :],
                                  x_re[:, 8 * h : 8 * (h + 1), :])

            # X^T [k, t] (fp32r: feeds the fp32r projections)
            xt = big.tile([128, BT], f32r, tag="xt")
            for n in range(NT):
                pt = ps_s.tile([128, 128], f32, tag="s")
                nc.tensor.transpose(pt[:], x_sb[:, n, :], ident[:])
                nc.vector.tensor_copy(xt[:, 128 * n : 128 * (n + 1)], pt[:])

            # projections (fp32r), evicted to bf16 for the attention matmuls
            qt = big.tile([128, BT], bf16, tag="qt")
            kt = big.tile([128, BT], bf16, tag="kt")
            vt = big.tile([128, BT], f32, tag="vt")
            for w_sb, dst in ((wq_sb, qt), (wk_sb, kt), (wv_sb, vt)):
                for blk in range(BT // 512):
                    pp = ps_s.tile([128, 512], f32, tag="s")
                    nc.tensor.matmul(
                        pp[:],
                        w_sb[:],
                        xt[:, 512 * blk : 512 * (blk + 1)],
                        start=True,
                        stop=True,
                    )
                    nc.vector.tensor_copy(dst[:, 512 * blk : 512 * (blk + 1)], pp[:])

            # V back to [s, j] layout (bf16), chunk c at cols c*128
            v_sb = big.tile([128, BT], bf16, tag="v")
            for c in range(NT):
                pt = ps_s.tile([128, 128], f32, tag="s")
                nc.tensor.transpose(pt[:], vt[:, 128 * c : 128 * (c + 1)], ident[:])
                nc.vector.tensor_copy(v_sb[:, 128 * c : 128 * (c + 1)], pt[:])

            # attention
            y_sb = big.tile([128, BT], f32r, tag="y")
            for b in range(B):
                for tb in range(T // TB):
                    tcol = b * T + tb * TB
                    py = ps_y.tile([128, TB], f32, tag="y")
                    psumt = ps_sum.tile([128, TB], f32, tag="sum")
                    for s in range(NS):
                        scol = b * T + s * 128
                        ps = ps_s.tile([128, TB], f32, tag="s")
                        for g in range(TB // 512):
                            sl = slice(512 * g, 512 * (g + 1))
                            nc.tensor.matmul(
                                ps[:, sl],
                                kt[:, scol : scol + 128],
                                qt[:, tcol + 512 * g : tcol + 512 * (g + 1)],
                                start=True,
                                stop=True,
                            )
                        e_sb = work.tile([128, TB], bf16, tag="e")
                        nc.scalar.activation(e_sb[:], ps[:], Exp, scale=float(SCALE))
                        for g in range(TB // 512):
                            sl = slice(512 * g, 512 * (g + 1))
                            nc.tensor.matmul(
                                psumt[:, sl],
                                ones[:],
                                e_sb[:, sl],
                                start=(s == 0),
                                stop=(s == NS - 1),
                                skip_group_check=True,
                            )
                            nc.tensor.matmul(
                                py[:, sl],
                                v_sb[:, scol : scol + 128],
                                e_sb[:, sl],
                                start=(s == 0),
                                stop=(s == NS - 1),
                                skip_group_check=True,
                            )
                    # sumexp is in [2e2, 2e4] — safely inside the approx
                    # reciprocal's domain; ~18 bits is plenty for softmax
                    # normalization (the plain InstReciprocal is 6.5us here)
                    r_sb = work.tile([128, TB], f32, tag="r")
                    nc.vector.reciprocal_approx_fast(r_sb[:], psumt[:])
                    nc.vector.tensor_mul(y_sb[:, tcol : tcol + TB], py[:], r_sb[:])

            # unify: out^T = Wu_h^T @ Y^T  (fp32r)
            out_sb = big.tile([128, BT], f32, tag="out")
            for blk in range(BT // 512):
                po = ps_s.tile([128, 512], f32, tag="s")
                nc.tensor.matmul(
                    po[:],
                    wu_sb[:],
                    y_sb[:, 512 * blk : 512 * (blk + 1)],
                    start=True,
                    stop=True,
                )
                nc.vector.tensor_copy(out_sb[:, 512 * blk : 512 * (blk + 1)], po[:])
                nc.sync.dma_start(out_d[:, 512 * blk : 512 * (blk + 1)],
                                  out_sb[:, 512 * blk : 512 * (blk + 1)])

    nc.compile()
    return nc


def _get_nc():
    global _compiled
    if _compiled is None:
        _compiled = _build()
    return _compiled


def kernel(x, Wq, Wk, Wv, Wu, bu, **_run_kwargs):
    from concourse.bass_utils import run_bass_kernel_spmd

    nc = _get_nc()

    x = np.ascontiguousarray(np.asarray(x, dtype=np.float32).reshape(BT, K))
    Wq = np.asarray(Wq, dtype=np.float32)
    Wk = np.asarray(Wk, dtype=np.float32)
    Wv = np.asarray(Wv, dtype=np.float32)
    Wu = np.asarray(Wu, dtype=np.float32)
    bu = np.asarray(bu, dtype=np.float32)

    in_maps = []
    for c in range(NCORES):
        sl = slice(c * K, (c + 1) * K)
        in_maps.append(
            {
                "x": x,
                "wq": np.ascontiguousarray(Wq[:, sl]),
                "wk": np.ascontiguousarray(Wk[:, sl]),
                "wv": np.ascontiguousarray(Wv[:, sl]),
                "wu": np.ascontiguousarray(Wu[sl, :]),
            }
        )

    res = run_bass_kernel_spmd(nc, in_maps, list(range(NCORES)), **_run_kwargs)

    out = np.zeros((BT, K), dtype=np.float32)
    for c in range(NCORES):
        out += res.results[c]["out"].T
    out += bu[None, :]
    result = out.reshape(B, T, K)
    if _run_kwargs:
        return result, res
    return result


# revision 12
# speedup vs baseline: 1.2182x; 1.0599x over previous
"""Multi-head attention (B=2, T=2048, H=8, K=128) on 8 TRN2 NeuronCores.

Sharding: tensor-parallel over heads — core c owns head c for both batches.
Each core computes its head's attention output projected through its slice
of Wu (a partial sum over the unified dim); the host sums the 8 partials
and adds the bias.

Per-core dataflow (everything "transposed": features on partitions, tokens
on the moving/free axis). All big matmuls run in bf16 (fp32 PSUM
accumulation): measured on HW, bf16 matmuls stream at 1 cyc/row while
fp32r runs ~3.7 cyc/row. Softmax statistics, normalization and the final
output projection stay fp32(r).

  x_bf  = bf16(x)        [t-tiles, k]    DVE cast
  X^T   [k=128, t=4096]  bf16 PE transposes of 32 [128,128] tiles
  Q^T/K^T/V^T = W^T X^T  [128, 4096]     bf16 matmuls
  V     [s-chunks, j]    bf16 PE transposes of V^T
  per 1024-token block:
    per 128-key chunk s:
      S^T_s = K_s Q^T              [128, 1024] PSUM fp32
      E_s   = exp(S^T_s/sqrt(128)) ACT -> SBUF bf16
      sumexp += ones^T E_s         [128, 1024] PSUM (replicated over parts)
      Y^T   += V_s^T E_s           [128, 1024] PSUM
    Y^T_norm = Y^T * recip_approx(sumexp)   DVE -> SBUF fp32r
  out^T = Wu_h^T Y^T_norm   [o=128, 4096] fp32r -> DRAM

Host: out = sum_c out_c^T.T + bu, reshaped to (2, 2048, 128).
"""

import sys

import numpy as np

if "/opt/trn_rl_repo" not in sys.path:
    sys.path.insert(0, "/opt/trn_rl_repo")

B, T, K, H = 2, 2048, 128, 8
BT = B * T              # 4096 tokens over both batches
NT = BT // 128          # 32 token tiles of 128
NCORES = 8
TB = 1024               # token block (2 psum banks)
NS = T // 128           # 16 key chunks per batch
SCALE = 1.0 / np.sqrt(np.float32(K))

_compiled = None


def _build():
    import concourse.mybir as mybir
    import concourse.tile as tile
    from concourse import bacc
    from concourse.masks import make_identity

    f32 = mybir.dt.float32
    f32r = mybir.dt.float32r
    bf16 = mybir.dt.bfloat16
    Exp = mybir.ActivationFunctionType.Exp

    nc = bacc.Bacc(
        "TRN2",
        target_bir_lowering=False,
        debug=False,
        enable_asserts=False,
        num_devices=NCORES,
    )

    x_d = nc.dram_tensor("x", [BT, K], f32, kind="ExternalInput").ap()
    wq_d = nc.dram_tensor("wq", [K, K], f32, kind="ExternalInput").ap()
    wk_d = nc.dram_tensor("wk", [K, K], f32, kind="ExternalInput").ap()
    wv_d = nc.dram_tensor("wv", [K, K], f32, kind="ExternalInput").ap()
    wu_d = nc.dram_tensor("wu", [K, K], f32, kind="ExternalInput").ap()
    out_d = nc.dram_tensor("out", [K, BT], f32, kind="ExternalOutput").ap()

    with tile.TileContext(nc) as tc:
        from contextlib import ExitStack

        with ExitStack() as ctx:
            const = ctx.enter_context(tc.tile_pool(name="const", bufs=1))
            big = ctx.enter_context(tc.tile_pool(name="big", bufs=1))
            work = ctx.enter_context(tc.tile_pool(name="work", bufs=3))
            # PSUM budget (8 banks): s 2x[128,1024]f32 = 4, y 1x = 2, sum 1x = 2
            ps_s = ctx.enter_context(tc.tile_pool(name="ps_s", bufs=2, space="PSUM"))
            ps_y = ctx.enter_context(tc.tile_pool(name="ps_y", bufs=1, space="PSUM"))
            ps_sum = ctx.enter_context(tc.tile_pool(name="ps_sum", bufs=1, space="PSUM"))

            # x first: everything downstream gates on it
            x_sb = []
            x_re = x_d.rearrange("(n p) k -> p n k", p=128)
            for h in range(4):
                xc = big.tile([128, 8, 128], f32, tag=f"x{h}")
                nc.sync.dma_start(xc[:], x_re[:, 8 * h : 8 * (h + 1), :])
                x_sb.append(xc)

            ident = const.tile([128, 128], bf16)
            make_identity(nc, ident[:])
            ones = const.tile([128, 128], bf16)
            nc.gpsimd.memset(ones[:], 1.0)

            # weights: DMA fp32, DVE cast to matmul dtypes
            wq_st = const.tile([128, 128], f32, tag="wq_st")
            wk_st = const.tile([128, 128], f32, tag="wk_st")
            wv_st = const.tile([128, 128], f32, tag="wv_st")
            wu_st = const.tile([128, 128], f32, tag="wu_st")
            nc.sync.dma_start(wq_st[:], wq_d[:])
            nc.sync.dma_start(wk_st[:], wk_d[:])
            nc.sync.dma_start(wv_st[:], wv_d[:])
            nc.sync.dma_start(wu_st[:], wu_d[:])
            wq_sb = const.tile([128, 128], bf16, tag="wq")
            wk_sb = const.tile([128, 128], bf16, tag="wk")
            wv_sb = const.tile([128, 128], bf16, tag="wv")
            wu_sb = const.tile([128, 128], f32r, tag="wu")
            nc.vector.tensor_copy(wq_sb[:], wq_st[:])
            nc.vector.tensor_copy(wk_sb[:], wk_st[:])
            nc.vector.tensor_copy(wv_sb[:], wv_st[:])
            nc.vector.tensor_copy(wu_sb[:], wu_st[:])

            # bf16 copy of x for the transposes
            x_bf = []
            for h in range(4):
                xb = big.tile([128, 8, 128], bf16, tag=f"xb{h}")
                nc.vector.tensor_copy(xb[:], x_sb[h][:])
                x_bf.append(xb)

            # X^T [k, t] bf16
            xt = big.tile([128, BT], bf16, tag="xt")
            for n in range(NT):
                pt = ps_s.tile([128, 128], bf16, tag="s")
                nc.tensor.transpose(pt[:], x_bf[n // 8][:, n % 8, :], ident[:])
                nc.vector.tensor_copy(xt[:, 128 * n : 128 * (n + 1)], pt[:])

            # projections (bf16): Q^T, K^T, V^T [i, t]
            qt = big.tile([128, BT], bf16, tag="qt")
            kt = big.tile([128, BT], bf16, tag="kt")
            vt = big.tile([128, BT], bf16, tag="vt")
            for w_sb, dst in ((wq_sb, qt), (wk_sb, kt), (wv_sb, vt)):
                for blk in range(BT // 512):
                    pp = ps_s.tile([128, 512], f32, tag="s")
                    nc.tensor.matmul(
                        pp[:],
                        w_sb[:],
                        xt[:, 512 * blk : 512 * (blk + 1)],
                        start=True,
                        stop=True,
                    )
                    nc.vector.tensor_copy(dst[:, 512 * blk : 512 * (blk + 1)], pp[:])

            # V back to [s, j] layout (bf16), chunk c at cols c*128
            v_sb = big.tile([128, BT], bf16, tag="v")
            for c in range(NT):
                pt = ps_s.tile([128, 128], bf16, tag="s")
                nc.tensor.transpose(pt[:], vt[:, 128 * c : 128 * (c + 1)], ident[:])
                nc.vector.tensor_copy(v_sb[:, 128 * c : 128 * (c + 1)], pt[:])

            # attention
            y_sb = big.tile([128, BT], f32r, tag="y")
            for b in range(B):
                for tb in range(T // TB):
                    tcol = b * T + tb * TB
                    py = ps_y.tile([128, TB], f32, tag="y")
                    psumt = ps_sum.tile([128, TB], f32, tag="sum")
                    for s in range(NS):
                        scol = b * T + s * 128
                        ps = ps_s.tile([128, TB], f32, tag="s")
                        for g in range(TB // 512):
                            sl = slice(512 * g, 512 * (g + 1))
                            nc.tensor.matmul(
                                ps[:, sl],
                                kt[:, scol : scol + 128],
                                qt[:, tcol + 512 * g : tcol + 512 * (g + 1)],
                                start=True,
                                stop=True,
                            )
                        e_sb = work.tile([128, TB], bf16, tag="e")
                        nc.scalar.activation(e_sb[:], ps[:], Exp, scale=float(SCALE))
                        for g in range(TB // 512):
                            sl = slice(512 * g, 512 * (g + 1))
                            nc.tensor.matmul(
                                psumt[:, sl],
                                ones[:],
                                e_sb[:, sl],
                                start=(s == 0),
                                stop=(s == NS - 1),
                                skip_group_check=True,
                            )
                            nc.tensor.matmul(
                                py[:, sl],
                                v_sb[:, scol : scol + 128],
                                e_sb[:, sl],
                                start=(s == 0),
                                stop=(s == NS - 1),
                                skip_group_check=True,
                            )
                    # sumexp is in [2e2, 2e4] — safely inside the approx
                    # reciprocal's domain; ~18 bits is plenty for softmax
                    # normalization (the exact InstReciprocal costs 6.5us)
                    r_sb = work.tile([128, TB], f32, tag="r")
                    nc.vector.reciprocal_approx_fast(r_sb[:], psumt[:])
                    nc.vector.tensor_mul(y_sb[:, tcol : tcol + TB], py[:], r_sb[:])

            # unify: out^T = Wu_h^T @ Y^T  (fp32r to protect the output)
            out_sb = big.tile([128, BT], f32, tag="out")
            for blk in range(BT // 512):
                po = ps_s.tile([128, 512], f32, tag="s")
                nc.tensor.matmul(
                    po[:],
                    wu_sb[:],
                    y_sb[:, 512 * blk : 512 * (blk + 1)],
                    start=True,
                    stop=True,
                )
                nc.vector.tensor_copy(out_sb[:, 512 * blk : 512 * (blk + 1)], po[:])
                nc.sync.dma_start(out_d[:, 512 * blk : 512 * (blk + 1)],
                                  out_sb[:, 512 * blk : 512 * (blk + 1)])

    nc.compile()
    return nc


def _get_nc():
    global _compiled
    if _compiled is None:
        _compiled = _build()
    return _compiled


def kernel(x, Wq, Wk, Wv, Wu, bu, **_run_kwargs):
    from concourse.bass_utils import run_bass_kernel_spmd

    nc = _get_nc()

    x = np.ascontiguousarray(np.asarray(x, dtype=np.float32).reshape(BT, K))
    Wq = np.asarray(Wq, dtype=np.float32)
    Wk = np.asarray(Wk, dtype=np.float32)
    Wv = np.asarray(Wv, dtype=np.float32)
    Wu = np.asarray(Wu, dtype=np.float32)
    bu = np.asarray(bu, dtype=np.float32)

    in_maps = []
    for c in range(NCORES):
        sl = slice(c * K, (c + 1) * K)
        in_maps.append(
            {
                "x": x,
                "wq": np.ascontiguousarray(Wq[:, sl]),
                "wk": np.ascontiguousarray(Wk[:, sl]),
                "wv": np.ascontiguousarray(Wv[:, sl]),
                "wu": np.ascontiguousarray(Wu[sl, :]),
            }
        )

    res = run_bass_kernel_spmd(nc, in_maps, list(range(NCORES)), **_run_kwargs)

    out = np.zeros((BT, K), dtype=np.float32)
    for c in range(NCORES):
        out += res.results[c]["out"].T
    out += bu[None, :]
    result = out.reshape(B, T, K)
    if _run_kwargs:
        return result, res
    return result


# revision 15
# speedup vs baseline: 1.2716x; 1.0438x over previous
"""Multi-head attention (B=2, T=2048, H=8, K=128) on 8 TRN2 NeuronCores.

Sharding: tensor-parallel over heads — core c owns head c for both batches.
Each core computes its head's attention output projected through its slice
of Wu (a partial sum over the unified dim); the host sums the 8 partials
and adds the bias.

Per-core dataflow (everything "transposed": features on partitions, tokens
on the moving/free axis). All big matmuls run in bf16 with fp32 PSUM
accumulation (bf16 streams at 1 cyc/row on the PE; fp32r measures ~3.7).
Softmax statistics, normalization and the output projection stay fp32(r).

  x_bf  = bf16(x)        [t-tiles, k]    DVE cast
  X^T   [k=128, t=4096]  bf16 PE transposes of 32 [128,128] tiles
  Q^T/K^T/V^T = W^T X^T  [128, 4096]     bf16 matmuls (V,K,Q interleaved)
  V     [s-chunks, j]    bf16 PE transposes of V^T
  per 1024-token block (software-pipelined over 128-key chunks s):
      S^T_s = K_s Q^T              [128, 1024] PSUM fp32
      E_s   = exp(S^T_s/sqrt(128)) ACT -> SBUF bf16
      sumexp += ones^T E_s         [128, 1024] PSUM (replicated over parts)
      Y^T   += V_s^T E_s           [128, 1024] PSUM
    Y^T_norm = Y^T * recip_approx(sumexp)   DVE -> SBUF fp32r
  out^T = Wu_h^T Y^T_norm   [o=128, 4096] fp32r -> DRAM

All large SBUF tensors are chunked into [128, 1024] tiles so phases
overlap at chunk granularity instead of serializing on whole-tensor deps.

Host: out = sum_c out_c^T.T + bu, reshaped to (2, 2048, 128).
"""

import sys

import numpy as np

if "/opt/trn_rl_repo" not in sys.path:
    sys.path.insert(0, "/opt/trn_rl_repo")

B, T, K, H = 2, 2048, 128, 8
BT = B * T              # 4096 tokens over both batches
NT = BT // 128          # 32 token tiles of 128
NC = BT // 1024         # 4 column chunks for the big SBUF tensors
NCORES = 8
TB = 1024               # token block (2 psum banks)
NS = T // 128           # 16 key chunks per batch
SCALE = 1.0 / np.sqrt(np.float32(K))

_compiled = None


def _build():
    import concourse.mybir as mybir
    import concourse.tile as tile
    from concourse import bacc
    from concourse.masks import make_identity

    f32 = mybir.dt.float32
    f32r = mybir.dt.float32r
    bf16 = mybir.dt.bfloat16
    Exp = mybir.ActivationFunctionType.Exp

    nc = bacc.Bacc(
        "TRN2",
        target_bir_lowering=False,
        debug=False,
        enable_asserts=False,
        num_devices=NCORES,
    )

    x_d = nc.dram_tensor("x", [BT, K], f32, kind="ExternalInput").ap()
    wq_d = nc.dram_tensor("wq", [K, K], f32, kind="ExternalInput").ap()
    wk_d = nc.dram_tensor("wk", [K, K], f32, kind="ExternalInput").ap()
    wv_d = nc.dram_tensor("wv", [K, K], f32, kind="ExternalInput").ap()
    wu_d = nc.dram_tensor("wu", [K, K], f32, kind="ExternalInput").ap()
    out_d = nc.dram_tensor("out", [K, BT], f32, kind="ExternalOutput").ap()

    with tile.TileContext(nc) as tc:
        from contextlib import ExitStack

        with ExitStack() as ctx:
            const = ctx.enter_context(tc.tile_pool(name="const", bufs=1))
            big = ctx.enter_context(tc.tile_pool(name="big", bufs=1))
            work = ctx.enter_context(tc.tile_pool(name="work", bufs=3))
            # PSUM budget (8 banks): s 2x[128,1024]f32 = 4, y 1x = 2, sum 1x = 2
            ps_s = ctx.enter_context(tc.tile_pool(name="ps_s", bufs=2, space="PSUM"))
            ps_y = ctx.enter_context(tc.tile_pool(name="ps_y", bufs=1, space="PSUM"))
            ps_sum = ctx.enter_context(tc.tile_pool(name="ps_sum", bufs=1, space="PSUM"))

            def chunked(tag, dtype):
                return [big.tile([128, 1024], dtype, tag=f"{tag}{c}",
                                 name=f"{tag}{c}")
                        for c in range(NC)]

            def cc(chunks, col, width):
                c, off = divmod(col, 1024)
                return chunks[c][:, off : off + width]

            # x first: everything downstream gates on it
            x_sb = []
            x_re = x_d.rearrange("(n p) k -> p n k", p=128)
            for h in range(8):
                xc = big.tile([128, 4, 128], f32, tag=f"x{h}")
                nc.sync.dma_start(xc[:], x_re[:, 4 * h : 4 * (h + 1), :])
                x_sb.append(xc)

            ident = const.tile([128, 128], bf16)
            make_identity(nc, ident[:])
            ones = const.tile([128, 128], bf16)
            nc.gpsimd.memset(ones[:], 1.0)

            # weights: DMA fp32, DVE cast to matmul dtypes
            wq_st = const.tile([128, 128], f32, tag="wq_st")
            wk_st = const.tile([128, 128], f32, tag="wk_st")
            wv_st = const.tile([128, 128], f32, tag="wv_st")
            wu_st = const.tile([128, 128], f32, tag="wu_st")
            nc.sync.dma_start(wv_st[:], wv_d[:])
            nc.sync.dma_start(wk_st[:], wk_d[:])
            nc.sync.dma_start(wq_st[:], wq_d[:])
            nc.sync.dma_start(wu_st[:], wu_d[:])
            wq_sb = const.tile([128, 128], bf16, tag="wq")
            wk_sb = const.tile([128, 128], bf16, tag="wk")
            wv_sb = const.tile([128, 128], bf16, tag="wv")
            wu_sb = const.tile([128, 128], f32r, tag="wu")
            nc.vector.tensor_copy(wv_sb[:], wv_st[:])
            nc.vector.tensor_copy(wk_sb[:], wk_st[:])
            nc.vector.tensor_copy(wq_sb[:], wq_st[:])
            nc.vector.tensor_copy(wu_sb[:], wu_st[:])

            # bf16 copy of x for the transposes
            x_bf = []
            for h in range(8):
                xb = big.tile([128, 4, 128], bf16, tag=f"xb{h}")
                nc.vector.tensor_copy(xb[:], x_sb[h][:])
                x_bf.append(xb)

            # X^T [k, t] bf16
            xt_c = chunked("xt", bf16)
            for n in range(NT):
                pt = ps_s.tile([128, 128], bf16, tag="s")
                nc.tensor.transpose(pt[:], x_bf[n // 4][:, n % 4, :], ident[:])
                nc.vector.tensor_copy(cc(xt_c, 128 * n, 128), pt[:])

            # projections (bf16), V first and interleaved so V-transposes and
            # attention start as early as possible
            qt_c = chunked("qt", bf16)
            kt_c = chunked("kt", bf16)
            vt_c = chunked("vt", bf16)
            v_c = chunked("v", bf16)
            for blk in range(BT // 512):
                for w_sb, dst in ((wv_sb, vt_c), (wk_sb, kt_c), (wq_sb, qt_c)):
                    pp = ps_s.tile([128, 512], f32, tag="s")
                    nc.tensor.matmul(
                        pp[:],
                        w_sb[:],
                        cc(xt_c, 512 * blk, 512),
                        start=True,
                        stop=True,
                    )
                    nc.vector.tensor_copy(cc(dst, 512 * blk, 512), pp[:])
                # V chunks of this 512-block back to [s, j] layout
                for c in range(4 * blk, 4 * blk + 4):
                    pt = ps_s.tile([128, 128], bf16, tag="s")
                    nc.tensor.transpose(pt[:], cc(vt_c, 128 * c, 128), ident[:])
                    nc.vector.tensor_copy(cc(v_c, 128 * c, 128), pt[:])

            # attention, software-pipelined: S for key-chunk s+1 is emitted
            # (and scheduled) ahead of the consumers of chunk s
            y_c = chunked("y", f32r)
            for b in range(B):
                for tb in range(T // TB):
                    tcol = b * T + tb * TB
                    py = ps_y.tile([128, TB], f32, tag="y")
                    psumt = ps_sum.tile([128, TB], f32, tag="sum")

                    def s_matmul(s):
                        scol = b * T + s * 128
                        ps = ps_s.tile([128, TB], f32, tag="s")
                        for g in range(TB // 512):
                            nc.tensor.matmul(
                                ps[:, 512 * g : 512 * (g + 1)],
                                cc(kt_c, scol, 128),
                                cc(qt_c, tcol + 512 * g, 512),
                                start=True,
                                stop=True,
                            )
                        return ps

                    pending = s_matmul(0)
                    for s in range(NS):
                        ps = pending
                        if s + 1 < NS:
                            pending = s_matmul(s + 1)
                        scol = b * T + s * 128
                        e_sb = work.tile([128, TB], bf16, tag="e")
                        nc.scalar.activation(e_sb[:], ps[:], Exp, scale=float(SCALE))
                        for g in range(TB // 512):
                            sl = slice(512 * g, 512 * (g + 1))
                            nc.tensor.matmul(
                                psumt[:, sl],
                                ones[:],
                                e_sb[:, sl],
                                start=(s == 0),
                                stop=(s == NS - 1),
                                skip_group_check=True,
                            )
                            nc.tensor.matmul(
                                py[:, sl],
                                cc(v_c, scol, 128),
                                e_sb[:, sl],
                                start=(s == 0),
                                stop=(s == NS - 1),
                                skip_group_check=True,
                            )
                    # sumexp is in [2e2, 2e4] — safely inside the approx
                    # reciprocal's domain; ~18 bits is plenty for softmax
                    # normalization (the exact InstReciprocal costs 6.5us)
                    r_sb = work.tile([128, TB], f32, tag="r")
                    nc.vector.reciprocal_approx_fast(r_sb[:], psumt[:])
                    nc.vector.tensor_mul(cc(y_c, tcol, TB), py[:], r_sb[:])

                    # unify this block: out^T = Wu_h^T @ Y^T (fp32r)
                    out_sb = big.tile([128, TB], f32, tag=f"out{tcol // TB}")
                    for g in range(TB // 512):
                        po = ps_s.tile([128, 512], f32, tag="s")
                        nc.tensor.matmul(
                            po[:],
                            wu_sb[:],
                            cc(y_c, tcol + 512 * g, 512),
                            start=True,
                            stop=True,
                        )
                        nc.vector.tensor_copy(out_sb[:, 512 * g : 512 * (g + 1)],
                                              po[:])
                    nc.sync.dma_start(out_d[:, tcol : tcol + TB], out_sb[:])

    nc.compile()
    return nc


def _get_nc():
    global _compiled
    if _compiled is None:
        _compiled = _build()
    return _compiled


def kernel(x, Wq, Wk, Wv, Wu, bu, **_run_kwargs):
    from concourse.bass_utils import run_bass_kernel_spmd

    nc = _get_nc()

    x = np.ascontiguousarray(np.asarray(x, dtype=np.float32).reshape(BT, K))
    Wq = np.asarray(Wq, dtype=np.float32)
    Wk = np.asarray(Wk, dtype=np.float32)
    Wv = np.asarray(Wv, dtype=np.float32)
    Wu = np.asarray(Wu, dtype=np.float32)
    bu = np.asarray(bu, dtype=np.float32)

    in_maps = []
    for c in range(NCORES):
        sl = slice(c * K, (c + 1) * K)
        in_maps.append(
            {
                "x": x,
                "wq": np.ascontiguousarray(Wq[:, sl]),
                "wk": np.ascontiguousarray(Wk[:, sl]),
                "wv": np.ascontiguousarray(Wv[:, sl]),
                "wu": np.ascontiguousarray(Wu[sl, :]),
            }
        )

    res = run_bass_kernel_spmd(nc, in_maps, list(range(NCORES)), **_run_kwargs)

    out = np.zeros((BT, K), dtype=np.float32)
    for c in range(NCORES):
        out += res.results[c]["out"].T
    out += bu[None, :]
    result = out.reshape(B, T, K)
    if _run_kwargs:
        return result, res
    return result


# revision 17
# speedup vs baseline: 1.3375x; 1.0518x over previous
"""Multi-head attention (B=2, T=2048, H=8, K=128) on 8 TRN2 NeuronCores.

Sharding: tensor-parallel over heads — core c owns head c for both batches.
Each core computes its head's attention output projected through its slice
of Wu (a partial sum over the unified dim); the host sums the 8 partials
and adds the bias.

Per-core dataflow (everything "transposed": features on partitions, tokens
on the moving/free axis). All big matmuls run in bf16 with fp32 PSUM
accumulation (bf16 streams at 1 cyc/row on the PE; fp32r measures ~3.7).
Softmax statistics, normalization and the output projection stay fp32(r).

  x_bf  = bf16(x)        [t-tiles, k]    DVE cast
  X^T   [k=128, t=4096]  bf16 PE transposes of 32 [128,128] tiles
  Q^T/K^T/V^T = W^T X^T  [128, 4096]     bf16 matmuls (V,K,Q interleaved)
  V     [s-chunks, j]    bf16 PE transposes of V^T
  per 1024-token block (software-pipelined over 128-key chunks s):
      S^T_s = K_s Q^T              [128, 1024] PSUM fp32
      E_s   = exp(S^T_s/sqrt(128)) ACT -> SBUF bf16
      sumexp += ones^T E_s         [128, 1024] PSUM (replicated over parts)
      Y^T   += V_s^T E_s           [128, 1024] PSUM
    Y^T_norm = Y^T * recip_approx(sumexp)   DVE -> SBUF fp32r
  out^T = Wu_h^T Y^T_norm   [o=128, 4096] fp32r -> DRAM

All large SBUF tensors are chunked into [128, 1024] tiles so phases
overlap at chunk granularity instead of serializing on whole-tensor deps.

Host: out = sum_c out_c^T.T + bu, reshaped to (2, 2048, 128).
"""

import sys

import numpy as np

if "/opt/trn_rl_repo" not in sys.path:
    sys.path.insert(0, "/opt/trn_rl_repo")

B, T, K, H = 2, 2048, 128, 8
BT = B * T              # 4096 tokens over both batches
NT = BT // 128          # 32 token tiles of 128
NC = BT // 1024         # 4 column chunks for the big SBUF tensors
NCORES = 8
TB = 1024               # token block (2 psum banks)
NS = T // 128           # 16 key chunks per batch
SCALE = 1.0 / np.sqrt(np.float32(K))

_compiled = None


def _build():
    import concourse.mybir as mybir
    import concourse.tile as tile
    from concourse import bacc
    from concourse.masks import make_identity

    f32 = mybir.dt.float32
    f32r = mybir.dt.float32r
    bf16 = mybir.dt.bfloat16
    Exp = mybir.ActivationFunctionType.Exp

    nc = bacc.Bacc(
        "TRN2",
        target_bir_lowering=False,
        debug=False,
        enable_asserts=False,
        num_devices=NCORES,
    )

    x_d = nc.dram_tensor("x", [BT, K], f32, kind="ExternalInput").ap()
    wq_d = nc.dram_tensor("wq", [K, K], f32, kind="ExternalInput").ap()
    wk_d = nc.dram_tensor("wk", [K, K], f32, kind="ExternalInput").ap()
    wv_d = nc.dram_tensor("wv", [K, K], f32, kind="ExternalInput").ap()
    wu_d = nc.dram_tensor("wu", [K, K], f32, kind="ExternalInput").ap()
    out_d = nc.dram_tensor("out", [K, BT], f32, kind="ExternalOutput").ap()

    with tile.TileContext(nc) as tc:
        from contextlib import ExitStack

        with ExitStack() as ctx:
            const = ctx.enter_context(tc.tile_pool(name="const", bufs=1))
            big = ctx.enter_context(tc.tile_pool(name="big", bufs=1))
            work = ctx.enter_context(tc.tile_pool(name="work", bufs=3))
            # PSUM budget (8 banks): s 2x[128,1024]f32 = 4, y 1x = 2, sum 1x = 2
            ps_s = ctx.enter_context(tc.tile_pool(name="ps_s", bufs=2, space="PSUM"))
            ps_y = ctx.enter_context(tc.tile_pool(name="ps_y", bufs=1, space="PSUM"))
            ps_sum = ctx.enter_context(tc.tile_pool(name="ps_sum", bufs=1, space="PSUM"))

            def chunked(tag, dtype):
                return [big.tile([128, 1024], dtype, tag=f"{tag}{c}",
                                 name=f"{tag}{c}")
                        for c in range(NC)]

            def cc(chunks, col, width):
                c, off = divmod(col, 1024)
                return chunks[c][:, off : off + width]

            # x first: everything downstream gates on it
            x_sb = []
            x_re = x_d.rearrange("(n p) k -> p n k", p=128)
            for h in range(8):
                xc = big.tile([128, 4, 128], f32, tag=f"x{h}")
                nc.sync.dma_start(xc[:], x_re[:, 4 * h : 4 * (h + 1), :])
                x_sb.append(xc)

            ident = const.tile([128, 128], bf16)
            make_identity(nc, ident[:])
            ones = const.tile([128, 128], bf16)
            nc.gpsimd.memset(ones[:], 1.0)

            # weights: DMA fp32, DVE cast to matmul dtypes
            wq_st = const.tile([128, 128], f32, tag="wq_st")
            wk_st = const.tile([128, 128], f32, tag="wk_st")
            wv_st = const.tile([128, 128], f32, tag="wv_st")
            wu_st = const.tile([128, 128], f32, tag="wu_st")
            nc.sync.dma_start(wv_st[:], wv_d[:])
            nc.sync.dma_start(wk_st[:], wk_d[:])
            nc.sync.dma_start(wq_st[:], wq_d[:])
            nc.sync.dma_start(wu_st[:], wu_d[:])
            wq_sb = const.tile([128, 128], bf16, tag="wq")
            wk_sb = const.tile([128, 128], bf16, tag="wk")
            wv_sb = const.tile([128, 128], bf16, tag="wv")
            wu_sb = const.tile([128, 128], bf16, tag="wu")
            nc.vector.tensor_copy(wv_sb[:], wv_st[:])
            nc.vector.tensor_copy(wk_sb[:], wk_st[:])
            nc.vector.tensor_copy(wq_sb[:], wq_st[:])
            nc.vector.tensor_copy(wu_sb[:], wu_st[:])

            # bf16 copy of x for the transposes
            x_bf = []
            for h in range(8):
                xb = big.tile([128, 4, 128], bf16, tag=f"xb{h}")
                nc.vector.tensor_copy(xb[:], x_sb[h][:])
                x_bf.append(xb)

            # X^T [k, t] bf16
            xt_c = chunked("xt", bf16)
            for n in range(NT):
                pt = ps_s.tile([128, 128], bf16, tag="s")
                nc.tensor.transpose(pt[:], x_bf[n // 4][:, n % 4, :], ident[:])
                nc.vector.tensor_copy(cc(xt_c, 128 * n, 128), pt[:])

            # projections (bf16), V first and interleaved so V-transposes and
            # attention start as early as possible
            qt_c = chunked("qt", bf16)
            kt_c = chunked("kt", bf16)
            vt_c = chunked("vt", bf16)
            v_c = chunked("v", bf16)
            for blk in range(BT // 512):
                for w_sb, dst in ((wv_sb, vt_c), (wk_sb, kt_c), (wq_sb, qt_c)):
                    pp = ps_s.tile([128, 512], f32, tag="s")
                    nc.tensor.matmul(
                        pp[:],
                        w_sb[:],
                        cc(xt_c, 512 * blk, 512),
                        start=True,
                        stop=True,
                    )
                    nc.vector.tensor_copy(cc(dst, 512 * blk, 512), pp[:])
                # V chunks of this 512-block back to [s, j] layout
                for c in range(4 * blk, 4 * blk + 4):
                    pt = ps_s.tile([128, 128], bf16, tag="s")
                    nc.tensor.transpose(pt[:], cc(vt_c, 128 * c, 128), ident[:])
                    nc.vector.tensor_copy(cc(v_c, 128 * c, 128), pt[:])

            # attention, software-pipelined ACROSS token blocks: the S
            # matmul for key-chunk s+1 (or the next block's chunk 0) is
            # emitted ahead of the consumers of chunk s, so the PE always
            # has independent work while exp runs / psum slots recycle
            y_c = chunked("y", bf16)
            blocks = [(b, tb) for b in range(B) for tb in range(T // TB)]

            def s_matmul(blk_i, s):
                b, tb = blocks[blk_i]
                scol = b * T + s * 128
                tcol = b * T + tb * TB
                ps = ps_s.tile([128, TB], f32, tag="s", name=f"ps_{blk_i}_{s}")
                for g in range(TB // 512):
                    nc.tensor.matmul(
                        ps[:, 512 * g : 512 * (g + 1)],
                        cc(kt_c, scol, 128),
                        cc(qt_c, tcol + 512 * g, 512),
                        start=True,
                        stop=True,
                    )
                return ps

            pending = s_matmul(0, 0)
            for blk_i, (b, tb) in enumerate(blocks):
                tcol = b * T + tb * TB
                py = ps_y.tile([128, TB], f32, tag="y")
                psumt = ps_sum.tile([128, TB], f32, tag="sum")
                for s in range(NS):
                    ps = pending
                    if s + 1 < NS:
                        pending = s_matmul(blk_i, s + 1)
                    elif blk_i + 1 < len(blocks):
                        pending = s_matmul(blk_i + 1, 0)
                    scol = b * T + s * 128
                    e_sb = work.tile([128, TB], bf16, tag="e")
                    nc.scalar.activation(e_sb[:], ps[:], Exp, scale=float(SCALE))
                    for g in range(TB // 512):
                        sl = slice(512 * g, 512 * (g + 1))
                        nc.tensor.matmul(
                            psumt[:, sl],
                            ones[:],
                            e_sb[:, sl],
                            start=(s == 0),
                            stop=(s == NS - 1),
                            skip_group_check=True,
                        )
                        nc.tensor.matmul(
                            py[:, sl],
                            cc(v_c, scol, 128),
                            e_sb[:, sl],
                            start=(s == 0),
                            stop=(s == NS - 1),
                            skip_group_check=True,
                        )
                # sumexp is in [2e2, 2e4] — safely inside the approx
                # reciprocal's domain; ~18 bits is plenty for softmax
                # normalization (the exact InstReciprocal costs 6.5us)
                r_sb = work.tile([128, TB], f32, tag="r")
                nc.vector.reciprocal_approx_fast(r_sb[:], psumt[:])
                nc.vector.tensor_mul(cc(y_c, tcol, TB), py[:], r_sb[:])

                # unify this block: out^T = Wu_h^T @ Y^T (bf16)
                out_sb = big.tile([128, TB], f32, tag=f"out{tcol // TB}",
                                  name=f"out_sb{tcol // TB}")
                for g in range(TB // 512):
                    po = ps_s.tile([128, 512], f32, tag="s")
                    nc.tensor.matmul(
                        po[:],
                        wu_sb[:],
                        cc(y_c, tcol + 512 * g, 512),
                        start=True,
                        stop=True,
                    )
                    nc.vector.tensor_copy(out_sb[:, 512 * g : 512 * (g + 1)],
                                          po[:])
                nc.sync.dma_start(out_d[:, tcol : tcol + TB], out_sb[:])

    nc.compile()
    return nc


def _get_nc():
    global _compiled
    if _compiled is None:
        _compiled = _build()
    return _compiled


def kernel(x, Wq, Wk, Wv, Wu, bu, **_run_kwargs):
    from concourse.bass_utils import run_bass_kernel_spmd

    nc = _get_nc()

    x = np.ascontiguousarray(np.asarray(x, dtype=np.float32).reshape(BT, K))
    Wq = np.asarray(Wq, dtype=np.float32)
    Wk = np.asarray(Wk, dtype=np.float32)
    Wv = np.asarray(Wv, dtype=np.float32)
    Wu = np.asarray(Wu, dtype=np.float32)
    bu = np.asarray(bu, dtype=np.float32)

    in_maps = []
    for c in range(NCORES):
        sl = slice(c * K, (c + 1) * K)
        in_maps.append(
            {
                "x": x,
                "wq": np.ascontiguousarray(Wq[:, sl]),
                "wk": np.ascontiguousarray(Wk[:, sl]),
                "wv": np.ascontiguousarray(Wv[:, sl]),
                "wu": np.ascontiguousarray(Wu[sl, :]),
            }
        )

    res = run_bass_kernel_spmd(nc, in_maps, list(range(NCORES)), **_run_kwargs)

    out = np.zeros((BT, K), dtype=np.float32)
    for c in range(NCORES):
        out += res.results[c]["out"].T
    out += bu[None, :]
    result = out.reshape(B, T, K)
    if _run_kwargs:
        return result, res
    return result
